# revision 12
# baseline (speedup 1.0000x reference)
"""Segment mean-pool (BERT lattice embedding) Trainium2 Bass kernel.

Full-input contract: kernel(hidden[64,512,768] f32, word_ids[64,512] i32,
num_tokens=400) -> [64,400,768] f32.

Strategy: data-parallel over batch across 8 NeuronCores (8 samples each).
Per sample b the ragged segment mean  out[t] = mean_{s: wid[s]==t} hidden[s]
is computed as a matmul on the PE array:

    A_T[s, c] = (word_ids[b, s] == perm(c))      one-hot, built on-device
    psum[c, :] = sum_j A_T[j-chunk].T @ hidden[b, j-chunk]
    out[t, h] = psum[c, h] * recip[b, t]         recip = 1/max(count,1)

Word-axis layout: perm(c) = 4*(c%100) + c//100, i.e. psum chunk m
(columns [100m, 100m+100)) holds words t = 4p + m on psum partition p.
All four chunks of a sample land in one om tile [100, 4, H] whose DMA to
out[b] is 12 KB/partition contiguous (one descriptor per partition, no
ragged 400-row tail, one output DMA instruction per sample instead of
four).  The permutation comes for free out of the gpsimd iota pattern
[[1,4],[4,100]].

Dtypes: all-bf16 matmuls. The one-hot lhsT is bf16 (0/1 exact) and the
hidden activations are cast f32 -> bf16 on the DVE right after landing
(~0.4 us per [128, 768] chunk).  This halves the LDWEIGHTS time, which
is what actually paces the PE (~210 ns/matmul for 4-byte weights vs
~160 ns of moving-operand streaming), and walrus rejects mixed
32-bit x 16-bit matmuls so the moving side must be bf16 too.  bf16
rounding of the activations costs ~2e-3 relative error against a 2e-2
gate.  Per-word piece-count reciprocals are derived on host from the
128 KB word_ids index tensor — index-side preprocessing; all heavy data
stays on device.

DMA plan (kernel is HBM-bound: 12.6 MB in + 9.8 MB out per core at a
~415 GB/s practical per-core ceiling = ~54 us of unavoidable streaming):
  - one merged aux tensor (word ids + reciprocals, 256 B/partition) at
    the head of the sync ring — NOT two tiny-packet transfers that would
    clog a ring for ~16 us while the other ring hogs the SDMA engines;
  - all hidden prefetches on the sync ring (sample 0 split per j-chunk so
    the first accumulation starts as soon as chunk 0 lands);
  - outputs on the scalar ring, one DMA per sample, except the LAST
    sample which is split per m-chunk so the final write is 0.3 MB
    issued right after its scale, not 1.2 MB serialized after the whole
    sample's compute.
"""

import numpy as np

B, S, H, T = 64, 512, 768, 400
N_CORES = 8
B_LOC = B // N_CORES  # samples per core
P = 128
J = S // P  # contraction chunks per sample
N0 = 384  # h-chunk split: two equal psum banks, balances the scale engines
MW = 100  # words per psum chunk (T = 4 * MW, psum partition p holds t=4p+m)
NM = 4  # word chunks per sample

_CACHED = {}


def build_program():
    """Build + compile the single-core Bass program (same NEFF on all cores)."""
    import concourse.bass as bass  # noqa: F401
    import concourse.mybir as mybir
    import concourse.tile as tile
    from concourse import bacc

    nc = bacc.Bacc(
        "TRN2",
        target_bir_lowering=False,
        debug=False,
        enable_asserts=False,
        num_devices=N_CORES,
    )
    f32 = mybir.dt.float32
    bf16 = mybir.dt.bfloat16

    hidden_t = nc.dram_tensor("hidden", [B_LOC, S, H], f32, kind="ExternalInput").ap()
    # aux[p, b, 0:4] = word_ids[b, 128j+p] (fp32; values < 400 exact), the
    # per-partition scalar for piece-chunk j.  aux[p, b, 4:8] (p < 100) =
    # 1/max(count,1) for word t = 4p + m.  One chunky DMA instead of two
    # 128-byte-descriptor trickles.
    aux_t = nc.dram_tensor("aux_pb", [P, B_LOC, 2 * NM], f32, kind="ExternalInput").ap()
    out_t = nc.dram_tensor("out", [B_LOC, T, H], f32, kind="ExternalOutput").ap()

    with tile.TileContext(nc) as tc:
        with tc.tile_pool(name="const", bufs=1) as const_pool, \
             tc.tile_pool(name="hidf", bufs=5) as hidf_pool, \
             tc.tile_pool(name="hidb", bufs=B_LOC) as hidb_pool, \
             tc.tile_pool(name="aTp", bufs=3) as aT_pool, \
             tc.tile_pool(name="outp", bufs=5) as out_pool, \
             tc.tile_pool(name="psum", bufs=4, space="PSUM") as psum_pool:

            aux_sb = const_pool.tile([P, B_LOC, 2 * NM], f32, name="aux_sb")
            nc.sync.dma_start(out=aux_sb, in_=aux_t)

            # iota_t[p, c] = 4*(c % 100) + c // 100 on every partition: chunk
            # m's columns carry words t = 4p + m in psum-partition order.
            iota_t = const_pool.tile([P, T], f32, name="iota_t")
            nc.gpsimd.iota(
                iota_t,
                pattern=[[1, NM], [NM, MW]],
                base=0,
                channel_multiplier=0,
                allow_small_or_imprecise_dtypes=True,
            )

            # Prefetch the whole input shard up front (fits in SBUF) on the
            # sync ring; the scalar ring is reserved for the output stream.
            hidfs = []
            for b in range(B_LOC):
                hidf = hidf_pool.tile([P, J, H], f32, name=f"hidf{b}", tag="hidf")
                src = hidden_t[b].rearrange("(j p) h -> p j h", p=P)
                if b == 0:
                    # First sample split per j-chunk so the first accumulation
                    # can start as soon as chunk 0 lands.
                    for j in range(J):
                        nc.sync.dma_start(out=hidf[:, j, :], in_=src[:, j, :])
                else:
                    nc.sync.dma_start(out=hidf, in_=src)
                hidfs.append(hidf)

            for b in range(B_LOC):
                # f32 -> bf16 casts ride the otherwise-idle GPSIMD engine so
                # DVE keeps its bandwidth for one-hots + half the scales.
                hid = hidb_pool.tile([P, J, H], bf16, name=f"hid{b}", tag="hid")
                if b == 0:
                    for j in range(J):
                        nc.gpsimd.tensor_copy(hid[:, j, :], hidfs[b][:, j, :])
                else:
                    nc.gpsimd.tensor_copy(hid, hidfs[b])
                aT = aT_pool.tile([P, J, T], bf16, name="aT", tag="aT")
                for j in range(J):
                    nc.vector.tensor_scalar(
                        aT[:, j, :],
                        iota_t,
                        aux_sb[:, b, j : j + 1],
                        None,
                        op0=mybir.AluOpType.is_equal,
                    )
                om = out_pool.tile([MW, NM, H], f32, name="om", tag="om")
                dst = out_t[b].rearrange("(p m) h -> p m h", m=NM)
                for m in range(NM):
                    ps0 = psum_pool.tile([MW, N0], f32, name="ps0", tag="ps0")
                    ps1 = psum_pool.tile([MW, H - N0], f32, name="ps1", tag="ps1")
                    for j in range(J):
                        nc.tensor.matmul(
                            ps0,
                            aT[:, j, m * MW : (m + 1) * MW],
                            hid[:, j, 0:N0],
                            start=(j == 0),
                            stop=(j == J - 1),
                        )
                    for j in range(J):
                        nc.tensor.matmul(
                            ps1,
                            aT[:, j, m * MW : (m + 1) * MW],
                            hid[:, j, N0:H],
                            start=(j == 0),
                            stop=(j == J - 1),
                        )
                    rec = aux_sb[:MW, b, NM + m : NM + m + 1]
                    # out = psum * (1/count): ACT and DVE each take one half,
                    # both read PSUM directly (each ~0.7 us — the pair in
                    # parallel keeps up with the PE's ~1.3 us chunk cadence).
                    nc.scalar.mul(om[:, m, 0:N0], ps0, rec)
                    nc.vector.tensor_scalar_mul(om[:, m, N0:H], ps1, rec)
                    if b == B_LOC - 1:
                        # Last sample: stream each chunk as soon as it's
                        # scaled so the final write is small.
                        nc.scalar.dma_start(out=dst[:, m], in_=om[:, m])
                if b < B_LOC - 1:
                    # One output DMA per sample: psum partition p of chunk m
                    # is word t = 4p+m, so om[p] maps to out rows 4p..4p+3 —
                    # a 12 KB/partition contiguous write.
                    nc.scalar.dma_start(out=dst, in_=om)

    nc.compile()
    return nc


def _prep_in_maps(hidden, word_ids):
    hidden = np.ascontiguousarray(np.asarray(hidden), dtype=np.float32).reshape(B, S, H)
    wid = np.ascontiguousarray(np.asarray(word_ids), dtype=np.int32).reshape(B, S)

    # Per-word piece counts -> 1/max(count,1).
    counts = np.zeros((B, T), np.int64)
    rows = np.repeat(np.arange(B), S)
    np.add.at(counts, (rows, wid.reshape(-1)), 1)
    recip = (1.0 / np.maximum(counts, 1)).astype(np.float32)  # [B, T]

    in_maps = []
    for i in range(N_CORES):
        sl = slice(i * B_LOC, (i + 1) * B_LOC)
        hs = np.ascontiguousarray(hidden[sl])
        ws = wid[sl]
        aux = np.ones((P, B_LOC, 2 * NM), np.float32)
        # aux[p, b, j] = wid[b, 128j+p]
        aux[:, :, :NM] = ws.reshape(B_LOC, J, P).transpose(2, 0, 1)
        # aux[p, b, 4+m] = recip[b, 4p+m]  (p < 100)
        aux[:MW, :, NM:] = recip[sl].reshape(B_LOC, MW, NM).transpose(1, 0, 2)
        in_maps.append({"hidden": hs, "aux_pb": np.ascontiguousarray(aux)})
    return in_maps


def run(hidden, word_ids, trace=False, **trace_kwargs):
    from concourse import bass_utils

    if "nc" not in _CACHED:
        _CACHED["nc"] = build_program()
    nc = _CACHED["nc"]
    in_maps = _prep_in_maps(hidden, word_ids)
    res = bass_utils.run_bass_kernel_spmd(
        nc, in_maps, core_ids=list(range(N_CORES)), trace=trace, **trace_kwargs
    )
    out = np.concatenate([res.results[i]["out"] for i in range(N_CORES)], axis=0)
    return out.astype(np.float32, copy=False), res


def kernel(hidden, word_ids, num_tokens=None, **_unused):
    out, _ = run(hidden, word_ids, trace=False)
    return out


# revision 15
# speedup vs baseline: 1.6963x; 1.6963x over previous
"""Segment mean-pool (BERT lattice embedding) Trainium2 Bass kernel.

Full-input contract: kernel(hidden[64,512,768] f32, word_ids[64,512] i32,
num_tokens=400) -> [64,400,768] f32.

Strategy: data-parallel over batch across 8 NeuronCores (8 samples each).
Per sample b the ragged segment mean  out[t] = mean_{s: wid[s]==t} hidden[s]
is computed as a matmul on the PE array:

    A_T[s, c] = (word_ids[b, s] == perm(c))      one-hot, built on-device
    psum[c, :] = sum_j A_T[j-chunk].T @ hidden[b, j-chunk]
    out[t, h] = psum[c, h] * recip[b, t]         recip = 1/max(count,1)

Word-axis layout: perm(c) = 4*(c%100) + c//100, i.e. psum chunk m
(columns [100m, 100m+100)) holds words t = 4p + m on psum partition p.
All four chunks of a sample land in one om tile [100, 4, H] whose DMA to
out[b] is 12 KB/partition contiguous (one descriptor per partition, no
ragged 400-row tail, one output DMA instruction per sample instead of
four).  The permutation comes for free out of the gpsimd iota pattern
[[1,4],[4,100]].

Dtypes: all-fp16 matmuls. fp16 represents integers up to 2048 and the
one-hot 0/1 exactly, so the iota/word-id compare runs at DVE's 2x
16-bit rate, and fp16 rounding of the activations costs ~5e-4 relative
error against a 2e-2 gate. The hidden activations are cast f32 -> fp16
on the DVE right after landing. 16-bit weights halve the LDWEIGHTS
time, which is what actually paces the PE (~210 ns/matmul for 4-byte
weights vs ~165 ns for 2-byte). walrus rejects mixed 32x16-bit
matmuls, so the moving operand must be 16-bit too.

PSUM: one [100, 768] f32 tile per m-chunk spanning exactly two banks,
accumulated by two matmul chains split 512+256 on the bank boundary.
One full-width scale op per m-chunk (alternating ACT / DVE) instead of
two half-width ones — the ~0.3 us fixed overhead per PSUM-reading op is
what used to pace the drain.

DMA plan (kernel is HBM-bound: 12.6 MB in + 9.8 MB out per core at a
~415 GB/s practical per-core ceiling = ~54 us of unavoidable streaming):
  - one merged aux tensor (word ids + reciprocals, 256 B/partition) at
    the head of the sync ring — NOT tiny-packet transfers that clog a
    ring while the other ring hogs the SDMA engines;
  - all hidden prefetches on the sync ring (sample 0 split per j-chunk
    so the first accumulation starts as soon as chunk 0 lands);
  - outputs on the scalar ring, one DMA per sample, except the LAST
    sample which is split per m-chunk so the final write is 0.3 MB
    issued right after its scale, not 1.2 MB serialized after the whole
    sample's compute.
"""

import numpy as np

B, S, H, T = 64, 512, 768, 400
N_CORES = 8
B_LOC = B // N_CORES  # samples per core
P = 128
J = S // P  # contraction chunks per sample
N0 = 512  # h-split on the PSUM bank boundary: chains of N=512 and N=256
MW = 100  # words per psum chunk (T = 4 * MW, psum partition p holds t=4p+m)
NM = 4  # word chunks per sample

_CACHED = {}


def build_program():
    """Build + compile the single-core Bass program (same NEFF on all cores)."""
    import concourse.bass as bass  # noqa: F401
    import concourse.mybir as mybir
    import concourse.tile as tile
    from concourse import bacc

    nc = bacc.Bacc(
        "TRN2",
        target_bir_lowering=False,
        debug=False,
        enable_asserts=False,
        num_devices=N_CORES,
    )
    f32 = mybir.dt.float32
    f16 = mybir.dt.float16

    hidden_t = nc.dram_tensor("hidden", [B_LOC, S, H], f32, kind="ExternalInput").ap()
    # aux[p, b, 0:4] = word_ids[b, 128j+p] (fp32; values < 400 exact), the
    # per-partition scalar for piece-chunk j.  aux[p, b, 4:8] (p < 100) =
    # 1/max(count,1) for word t = 4p + m.  One chunky DMA instead of two
    # 128-byte-descriptor trickles.
    aux_t = nc.dram_tensor("aux_pb", [P, B_LOC, 2 * NM], f32, kind="ExternalInput").ap()
    out_t = nc.dram_tensor("out", [B_LOC, T, H], f32, kind="ExternalOutput").ap()

    with tile.TileContext(nc) as tc:
        with tc.tile_pool(name="const", bufs=1) as const_pool, \
             tc.tile_pool(name="hidf", bufs=5) as hidf_pool, \
             tc.tile_pool(name="hidb", bufs=B_LOC) as hidb_pool, \
             tc.tile_pool(name="aTp", bufs=3) as aT_pool, \
             tc.tile_pool(name="outp", bufs=5) as out_pool, \
             tc.tile_pool(name="psum", bufs=4, space="PSUM") as psum_pool:

            aux_sb = const_pool.tile([P, B_LOC, 2 * NM], f32, name="aux_sb")
            nc.sync.dma_start(out=aux_sb, in_=aux_t)

            # iota_t[p, c] = 4*(c % 100) + c // 100 on every partition: chunk
            # m's columns carry words t = 4p + m in psum-partition order.
            # fp16 is exact for integers < 2048, and 16-bit in/out runs the
            # one-hot compares at DVE's 2x rate.
            iota_t = const_pool.tile([P, T], f16, name="iota_t")
            nc.gpsimd.iota(
                iota_t,
                pattern=[[1, NM], [NM, MW]],
                base=0,
                channel_multiplier=0,
                allow_small_or_imprecise_dtypes=True,
            )
            # Prefetch the whole input shard up front (fits in SBUF) on the
            # sync ring; the scalar ring is reserved for the output stream.
            hidfs = []
            for b in range(B_LOC):
                hidf = hidf_pool.tile([P, J, H], f32, name=f"hidf{b}", tag="hidf")
                src = hidden_t[b].rearrange("(j p) h -> p j h", p=P)
                if b == 0:
                    # First sample split per j-chunk so the first accumulation
                    # can start as soon as chunk 0 lands.
                    for j in range(J):
                        nc.sync.dma_start(out=hidf[:, j, :], in_=src[:, j, :])
                else:
                    nc.sync.dma_start(out=hidf, in_=src)
                hidfs.append(hidf)

            for b in range(B_LOC):
                hid = hidb_pool.tile([P, J, H], f16, name=f"hid{b}", tag="hid")
                if b == 0:
                    for j in range(J):
                        nc.vector.tensor_copy(hid[:, j, :], hidfs[b][:, j, :])
                else:
                    nc.vector.tensor_copy(hid, hidfs[b])
                aT = aT_pool.tile([P, J, T], f16, name="aT", tag="aT")
                for j in range(J):
                    nc.vector.tensor_scalar(
                        aT[:, j, :],
                        iota_t,
                        aux_sb[:, b, j : j + 1],
                        None,
                        op0=mybir.AluOpType.is_equal,
                    )
                om = out_pool.tile([MW, NM, H], f32, name="om", tag="om")
                dst = out_t[b].rearrange("(p m) h -> p m h", m=NM)
                for m in range(NM):
                    ps = psum_pool.tile([MW, H], f32, name="ps", tag="ps")
                    for j in range(J):
                        nc.tensor.matmul(
                            ps[:, 0:N0],
                            aT[:, j, m * MW : (m + 1) * MW],
                            hid[:, j, 0:N0],
                            start=(j == 0),
                            stop=(j == J - 1),
                        )
                        nc.tensor.matmul(
                            ps[:, N0:H],
                            aT[:, j, m * MW : (m + 1) * MW],
                            hid[:, j, N0:H],
                            start=(j == 0),
                            stop=(j == J - 1),
                        )
                    rec = aux_sb[:MW, b, NM + m : NM + m + 1]
                    # out = psum * (1/count): one full-width op per m-chunk,
                    # alternating ACT / DVE so the two run in parallel.
                    if m % 2 == 0:
                        nc.scalar.mul(om[:, m, :], ps, rec)
                    else:
                        nc.vector.tensor_scalar_mul(om[:, m, :], ps, rec)
                    if b == B_LOC - 1:
                        # Last sample: stream each chunk as soon as it's
                        # scaled so the final write is small.
                        nc.scalar.dma_start(out=dst[:, m], in_=om[:, m])
                if b < B_LOC - 1:
                    # One output DMA per sample: psum partition p of chunk m
                    # is word t = 4p+m, so om[p] maps to out rows 4p..4p+3 —
                    # a 12 KB/partition contiguous write.
                    nc.scalar.dma_start(out=dst, in_=om)

    nc.compile()
    return nc


def _prep_in_maps(hidden, word_ids):
    hidden = np.ascontiguousarray(np.asarray(hidden), dtype=np.float32).reshape(B, S, H)
    wid = np.ascontiguousarray(np.asarray(word_ids), dtype=np.int32).reshape(B, S)

    # Per-word piece counts -> 1/max(count,1).
    counts = np.zeros((B, T), np.int64)
    rows = np.repeat(np.arange(B), S)
    np.add.at(counts, (rows, wid.reshape(-1)), 1)
    recip = (1.0 / np.maximum(counts, 1)).astype(np.float32)  # [B, T]

    in_maps = []
    for i in range(N_CORES):
        sl = slice(i * B_LOC, (i + 1) * B_LOC)
        hs = np.ascontiguousarray(hidden[sl])
        ws = wid[sl]
        aux = np.ones((P, B_LOC, 2 * NM), np.float32)
        # aux[p, b, j] = wid[b, 128j+p]
        aux[:, :, :NM] = ws.reshape(B_LOC, J, P).transpose(2, 0, 1)
        # aux[p, b, 4+m] = recip[b, 4p+m]  (p < 100)
        aux[:MW, :, NM:] = recip[sl].reshape(B_LOC, MW, NM).transpose(1, 0, 2)
        in_maps.append({"hidden": hs, "aux_pb": np.ascontiguousarray(aux)})
    return in_maps


def run(hidden, word_ids, trace=False, **trace_kwargs):
    from concourse import bass_utils

    if "nc" not in _CACHED:
        _CACHED["nc"] = build_program()
    nc = _CACHED["nc"]
    in_maps = _prep_in_maps(hidden, word_ids)
    res = bass_utils.run_bass_kernel_spmd(
        nc, in_maps, core_ids=list(range(N_CORES)), trace=trace, **trace_kwargs
    )
    out = np.concatenate([res.results[i]["out"] for i in range(N_CORES)], axis=0)
    return out.astype(np.float32, copy=False), res


def kernel(hidden, word_ids, num_tokens=None, **_unused):
    out, _ = run(hidden, word_ids, trace=False)
    return out


# revision 16
# speedup vs baseline: 1.8417x; 1.0857x over previous
"""Segment mean-pool (BERT lattice embedding) Trainium2 Bass kernel.

Full-input contract: kernel(hidden[64,512,768] f32, word_ids[64,512] i32,
num_tokens=400) -> [64,400,768] f32.

Strategy: data-parallel over batch across 8 NeuronCores (8 samples each).
Per sample b the ragged segment mean  out[t] = mean_{s: wid[s]==t} hidden[s]
is computed as a matmul on the PE array:

    A_T[s, c] = (word_ids[b, s] == perm(c))      one-hot, built on-device
    psum[c, :] = sum_j A_T[j-chunk].T @ hidden[b, j-chunk]
    out[t, h] = psum[c, h] * recip[b, t]         recip = 1/max(count,1)

Word-axis layout: perm(c) = 4*(c%100) + c//100, i.e. psum chunk m
(columns [100m, 100m+100)) holds words t = 4p + m on psum partition p.
All four chunks of a sample land in one om tile [100, 4, H] whose DMA to
out[b] is 12 KB/partition contiguous (one descriptor per partition, no
ragged 400-row tail, one output DMA instruction per sample instead of
four).  The permutation comes for free out of the gpsimd iota pattern
[[1,4],[4,100]].

Dtypes: all-fp16 matmuls (fp16 is exact for the 0/1 one-hot and for
integers < 2048, and rounds the activations at ~5e-4 relative error
against a 2e-2 gate).  16-bit weights halve the LDWEIGHTS time, which
is what actually paces the PE (~210 ns/matmul for 4-byte weights vs
~165 ns for 2-byte).  The f32 -> fp16 conversion happens INSIDE the
input DMA: SWDGE (gpsimd-ring) DMAs cast in the SDMA datapath at line
rate, so no compute engine spends a cycle on it and no f32 staging
SBUF is needed.  Piece layout s = 4p + j makes each sample's read one
12 KB-contiguous descriptor per partition — ~128 descriptors per
sample, cheap for the Q7 descriptor generator.

DMA plan (kernel is HBM-bound: 12.6 MB in + 9.8 MB out per core at a
~415 GB/s practical per-core ceiling = ~54 us of unavoidable streaming):
  - aux (word ids + reciprocals, one 256 B/partition transfer) alone at
    the head of the sync HWDGE ring;
  - the hidden stream rides SWDGE queues (sample 0 in two halves so the
    first accumulations start early);
  - outputs alternate between the two HWDGE rings (even samples ->
    scalar, odd -> sync) so the output-only drain at the end runs on
    both descriptor generators; the last two samples go per m-chunk so
    the final writes are 0.3 MB right after their scale, not 1.2 MB
    serialized after a whole sample's compute.
"""

import numpy as np

B, S, H, T = 64, 512, 768, 400
N_CORES = 8
B_LOC = B // N_CORES  # samples per core
P = 128
J = S // P  # contraction chunks per sample
N0 = 384  # h-chunk split: two equal psum banks, balances the scale engines
MW = 100  # words per psum chunk (T = 4 * MW, psum partition p holds t=4p+m)
NM = 4  # word chunks per sample

_CACHED = {}


def build_program():
    """Build + compile the single-core Bass program (same NEFF on all cores)."""
    import concourse.bass as bass  # noqa: F401
    import concourse.mybir as mybir
    import concourse.tile as tile
    from concourse import bacc

    nc = bacc.Bacc(
        "TRN2",
        target_bir_lowering=False,
        debug=False,
        enable_asserts=False,
        num_devices=N_CORES,
    )
    f32 = mybir.dt.float32
    f16 = mybir.dt.float16

    hidden_t = nc.dram_tensor("hidden", [B_LOC, S, H], f32, kind="ExternalInput").ap()
    # aux[p, b, 0:4] = word_ids[b, 4p+j] (fp32; values < 400 exact), the
    # per-partition scalar for piece-chunk j.  aux[p, b, 4:8] (p < 100) =
    # 1/max(count,1) for word t = 4p + m.
    aux_t = nc.dram_tensor("aux_pb", [P, B_LOC, 2 * NM], f32, kind="ExternalInput").ap()
    out_t = nc.dram_tensor("out", [B_LOC, T, H], f32, kind="ExternalOutput").ap()

    with tile.TileContext(nc) as tc:
        with tc.tile_pool(name="const", bufs=1) as const_pool, \
             tc.tile_pool(name="hidp", bufs=B_LOC) as hid_pool, \
             tc.tile_pool(name="aTp", bufs=3) as aT_pool, \
             tc.tile_pool(name="outp", bufs=6) as out_pool, \
             tc.tile_pool(name="psum", bufs=4, space="PSUM") as psum_pool:

            aux_sb = const_pool.tile([P, B_LOC, 2 * NM], f32, name="aux_sb")
            nc.sync.dma_start(out=aux_sb, in_=aux_t)

            # iota_t[p, c] = 4*(c % 100) + c // 100 on every partition: chunk
            # m's columns carry words t = 4p + m in psum-partition order.
            iota_t = const_pool.tile([P, T], f16, name="iota_t")
            nc.gpsimd.iota(
                iota_t,
                pattern=[[1, NM], [NM, MW]],
                base=0,
                channel_multiplier=0,
                allow_small_or_imprecise_dtypes=True,
            )

            # Input prefetch: SWDGE DMAs cast f32 -> fp16 inline.  Piece
            # s = 4p + j: partition p's four rows are consecutive in DRAM,
            # one 12 KB contiguous read per partition per sample.
            hids = []
            for b in range(B_LOC):
                hid = hid_pool.tile([P, J, H], f16, name=f"hid{b}", tag="hid")
                src = hidden_t[b].rearrange("(p j) h -> p j h", j=J)
                if b == 0:
                    # First sample in two halves (j pairs are still DRAM-
                    # contiguous per partition) so compute starts early.
                    nc.gpsimd.dma_start(out=hid[:, 0:2, :], in_=src[:, 0:2, :])
                    nc.gpsimd.dma_start(out=hid[:, 2:4, :], in_=src[:, 2:4, :])
                else:
                    nc.gpsimd.dma_start(out=hid, in_=src)
                hids.append(hid)

            for b in range(B_LOC):
                hid = hids[b]
                aT = aT_pool.tile([P, J, T], f16, name="aT", tag="aT")
                for j in range(J):
                    nc.vector.tensor_scalar(
                        aT[:, j, :],
                        iota_t,
                        aux_sb[:, b, j : j + 1],
                        None,
                        op0=mybir.AluOpType.is_equal,
                    )
                om = out_pool.tile([MW, NM, H], f32, name="om", tag="om")
                dst = out_t[b].rearrange("(p m) h -> p m h", m=NM)
                out_eng = nc.scalar if b % 2 == 0 else nc.sync
                for m in range(NM):
                    ps0 = psum_pool.tile([MW, N0], f32, name="ps0", tag="ps0")
                    ps1 = psum_pool.tile([MW, H - N0], f32, name="ps1", tag="ps1")
                    for j in range(J):
                        nc.tensor.matmul(
                            ps0,
                            aT[:, j, m * MW : (m + 1) * MW],
                            hid[:, j, 0:N0],
                            start=(j == 0),
                            stop=(j == J - 1),
                        )
                    for j in range(J):
                        nc.tensor.matmul(
                            ps1,
                            aT[:, j, m * MW : (m + 1) * MW],
                            hid[:, j, N0:H],
                            start=(j == 0),
                            stop=(j == J - 1),
                        )
                    rec = aux_sb[:MW, b, NM + m : NM + m + 1]
                    # out = psum * (1/count): ACT and DVE each take one half,
                    # both read PSUM directly.
                    nc.scalar.mul(om[:, m, 0:N0], ps0, rec)
                    nc.vector.tensor_scalar_mul(om[:, m, N0:H], ps1, rec)
                    if b >= B_LOC - 2:
                        # Last two samples: stream each chunk as soon as it's
                        # scaled so the final writes are small and the drain
                        # runs on both HWDGE rings.
                        out_eng.dma_start(out=dst[:, m], in_=om[:, m])
                if b < B_LOC - 2:
                    # One output DMA per sample: psum partition p of chunk m
                    # is word t = 4p+m, so om[p] maps to out rows 4p..4p+3 —
                    # a 12 KB/partition contiguous write.
                    out_eng.dma_start(out=dst, in_=om)

    nc.compile()
    return nc


def _prep_in_maps(hidden, word_ids):
    hidden = np.ascontiguousarray(np.asarray(hidden), dtype=np.float32).reshape(B, S, H)
    wid = np.ascontiguousarray(np.asarray(word_ids), dtype=np.int32).reshape(B, S)

    # Per-word piece counts -> 1/max(count,1).
    counts = np.zeros((B, T), np.int64)
    rows = np.repeat(np.arange(B), S)
    np.add.at(counts, (rows, wid.reshape(-1)), 1)
    recip = (1.0 / np.maximum(counts, 1)).astype(np.float32)  # [B, T]

    in_maps = []
    for i in range(N_CORES):
        sl = slice(i * B_LOC, (i + 1) * B_LOC)
        hs = np.ascontiguousarray(hidden[sl])
        ws = wid[sl]
        aux = np.ones((P, B_LOC, 2 * NM), np.float32)
        # aux[p, b, j] = wid[b, 4p+j]   (piece s = 4p + j)
        aux[:, :, :NM] = ws.reshape(B_LOC, P, J).transpose(1, 0, 2)
        # aux[p, b, 4+m] = recip[b, 4p+m]  (p < 100)
        aux[:MW, :, NM:] = recip[sl].reshape(B_LOC, MW, NM).transpose(1, 0, 2)
        in_maps.append({"hidden": hs, "aux_pb": np.ascontiguousarray(aux)})
    return in_maps


def run(hidden, word_ids, trace=False, **trace_kwargs):
    from concourse import bass_utils

    if "nc" not in _CACHED:
        _CACHED["nc"] = build_program()
    nc = _CACHED["nc"]
    in_maps = _prep_in_maps(hidden, word_ids)
    res = bass_utils.run_bass_kernel_spmd(
        nc, in_maps, core_ids=list(range(N_CORES)), trace=trace, **trace_kwargs
    )
    out = np.concatenate([res.results[i]["out"] for i in range(N_CORES)], axis=0)
    return out.astype(np.float32, copy=False), res


def kernel(hidden, word_ids, num_tokens=None, **_unused):
    out, _ = run(hidden, word_ids, trace=False)
    return out


# revision 17
# speedup vs baseline: 1.8491x; 1.0040x over previous
"""Segment mean-pool (BERT lattice embedding) Trainium2 Bass kernel.

Full-input contract: kernel(hidden[64,512,768] f32, word_ids[64,512] i32,
num_tokens=400) -> [64,400,768] f32.

Strategy: data-parallel over batch across 8 NeuronCores (8 samples each).
Per sample b the ragged segment mean  out[t] = mean_{s: wid[s]==t} hidden[s]
is computed as a matmul on the PE array:

    A_T[s, c] = (word_ids[b, s] == perm(c))      one-hot, built on-device
    psum[c, :] = sum_j A_T[j-chunk].T @ hidden[b, j-chunk]
    out[t, h] = psum[c, h] * recip[b, t]         recip = 1/max(count,1)

Word-axis layout: perm(c) = 4*(c%100) + c//100, i.e. psum chunk m
(columns [100m, 100m+100)) holds words t = 4p + m on psum partition p.
All four chunks of a sample land in one om tile [100, 4, H] whose DMA to
out[b] is 12 KB/partition contiguous — no ragged 400-row tail and one
output DMA instruction per sample instead of four.  The permutation
comes for free out of the gpsimd iota pattern [[1,4],[4,100]].

Dtypes: all-fp16 matmuls (fp16 is exact for the 0/1 one-hot and for
integers < 2048, and rounds the activations at ~4e-4 relative error
against a 2e-2 gate).  16-bit weights halve the LDWEIGHTS time, which
is what actually paces the PE (~210 ns/matmul for 4-byte weights vs
~163 ns measured for 2-byte).  The f32 -> fp16 casts are ordinary
engine copies spread across ACT and DVE so that, together with the
one-hots (DVE), the PSUM scales (split ACT/DVE) and the output-DMA
issues, every engine's total stays at ~42 us — just under the PE's
~43 us and the ~54 us HBM streaming floor.  (SWDGE could cast inside
the DMA, but its Q7 descriptor generator tops out ~7x too slow for
this stream; HWDGE + engine casts is faster end-to-end.)

DMA plan (kernel is HBM-bound: 12.6 MB in + 9.8 MB out per core at a
~415 GB/s practical per-core ceiling = ~54 us of unavoidable streaming):
  - one merged aux tensor (word ids + reciprocals, 256 B/partition) at
    the head of the sync ring — NOT tiny-packet transfers that would
    clog a ring while the other hogs the SDMA engines;
  - all hidden prefetches on the sync HWDGE ring, 3 KB descriptors
    (sample 0 split per j-chunk so compute starts with chunk 0);
  - outputs alternate rings (even samples -> scalar ring, odd -> sync
    ring, queued after the inputs there) so the output-only drain phase
    runs on both descriptor generators; the last two samples go per
    m-chunk so the final writes are 0.3 MB issued right after their
    scale, not 1.2 MB serialized after a whole sample's compute.
"""

import numpy as np

B, S, H, T = 64, 512, 768, 400
N_CORES = 8
B_LOC = B // N_CORES  # samples per core
P = 128
J = S // P  # contraction chunks per sample
N0 = 384  # h-chunk split: two equal psum banks, balances the scale engines
MW = 100  # words per psum chunk (T = 4 * MW, psum partition p holds t=4p+m)
NM = 4  # word chunks per sample

_CACHED = {}


def build_program():
    """Build + compile the single-core Bass program (same NEFF on all cores)."""
    import concourse.bass as bass  # noqa: F401
    import concourse.mybir as mybir
    import concourse.tile as tile
    from concourse import bacc

    nc = bacc.Bacc(
        "TRN2",
        target_bir_lowering=False,
        debug=False,
        enable_asserts=False,
        num_devices=N_CORES,
    )
    f32 = mybir.dt.float32
    f16 = mybir.dt.float16

    hidden_t = nc.dram_tensor("hidden", [B_LOC, S, H], f32, kind="ExternalInput").ap()
    # aux[p, b, 0:4] = word_ids[b, 128j+p] (fp32; values < 400 exact), the
    # per-partition scalar for piece-chunk j.  aux[p, b, 4:8] (p < 100) =
    # 1/max(count,1) for word t = 4p + m.
    aux_t = nc.dram_tensor("aux_pb", [P, B_LOC, 2 * NM], f32, kind="ExternalInput").ap()
    out_t = nc.dram_tensor("out", [B_LOC, T, H], f32, kind="ExternalOutput").ap()

    with tile.TileContext(nc) as tc:
        with tc.tile_pool(name="const", bufs=1) as const_pool, \
             tc.tile_pool(name="hidf", bufs=5) as hidf_pool, \
             tc.tile_pool(name="hidb", bufs=B_LOC) as hidb_pool, \
             tc.tile_pool(name="aTp", bufs=3) as aT_pool, \
             tc.tile_pool(name="outp", bufs=6) as out_pool, \
             tc.tile_pool(name="psum", bufs=4, space="PSUM") as psum_pool:

            aux_sb = const_pool.tile([P, B_LOC, 2 * NM], f32, name="aux_sb")
            nc.sync.dma_start(out=aux_sb, in_=aux_t)

            # iota_t[p, c] = 4*(c % 100) + c // 100 on every partition: chunk
            # m's columns carry words t = 4p + m in psum-partition order.
            iota_t = const_pool.tile([P, T], f16, name="iota_t")
            nc.gpsimd.iota(
                iota_t,
                pattern=[[1, NM], [NM, MW]],
                base=0,
                channel_multiplier=0,
                allow_small_or_imprecise_dtypes=True,
            )

            # Prefetch the whole f32 shard up front (fits in SBUF) on the
            # sync ring.
            hidfs = []
            for b in range(B_LOC):
                hidf = hidf_pool.tile([P, J, H], f32, name=f"hidf{b}", tag="hidf")
                src = hidden_t[b].rearrange("(j p) h -> p j h", p=P)
                if b == 0:
                    for j in range(J):
                        nc.sync.dma_start(out=hidf[:, j, :], in_=src[:, j, :])
                else:
                    nc.sync.dma_start(out=hidf, in_=src)
                hidfs.append(hidf)

            for b in range(B_LOC):
                hid = hidb_pool.tile([P, J, H], f16, name=f"hid{b}", tag="hid")
                # Cast f32 -> fp16.  Sample 0 per j-chunk on DVE (starts as
                # chunk 0 lands); sample 4 whole on DVE; the rest on ACT —
                # this is the engine-balance point.
                if b == 0:
                    for j in range(J):
                        nc.vector.tensor_copy(hid[:, j, :], hidfs[b][:, j, :])
                elif b == 4:
                    nc.vector.tensor_copy(hid, hidfs[b])
                else:
                    nc.scalar.copy(hid, hidfs[b])
                aT = aT_pool.tile([P, J, T], f16, name="aT", tag="aT")
                for j in range(J):
                    nc.vector.tensor_scalar(
                        aT[:, j, :],
                        iota_t,
                        aux_sb[:, b, j : j + 1],
                        None,
                        op0=mybir.AluOpType.is_equal,
                    )
                om = out_pool.tile([MW, NM, H], f32, name="om", tag="om")
                dst = out_t[b].rearrange("(p m) h -> p m h", m=NM)
                out_eng = nc.scalar if b % 2 == 0 else nc.sync
                for m in range(NM):
                    ps0 = psum_pool.tile([MW, N0], f32, name="ps0", tag="ps0")
                    ps1 = psum_pool.tile([MW, H - N0], f32, name="ps1", tag="ps1")
                    for j in range(J):
                        nc.tensor.matmul(
                            ps0,
                            aT[:, j, m * MW : (m + 1) * MW],
                            hid[:, j, 0:N0],
                            start=(j == 0),
                            stop=(j == J - 1),
                        )
                    for j in range(J):
                        nc.tensor.matmul(
                            ps1,
                            aT[:, j, m * MW : (m + 1) * MW],
                            hid[:, j, N0:H],
                            start=(j == 0),
                            stop=(j == J - 1),
                        )
                    rec = aux_sb[:MW, b, NM + m : NM + m + 1]
                    # out = psum * (1/count): ACT and DVE each take one half,
                    # both read PSUM directly.
                    nc.scalar.mul(om[:, m, 0:N0], ps0, rec)
                    nc.vector.tensor_scalar_mul(om[:, m, N0:H], ps1, rec)
                    if b >= B_LOC - 2:
                        # Last two samples: stream each chunk as soon as it's
                        # scaled so the final writes are small and the drain
                        # runs on both HWDGE rings.
                        out_eng.dma_start(out=dst[:, m], in_=om[:, m])
                if b < B_LOC - 2:
                    # One output DMA per sample: psum partition p of chunk m
                    # is word t = 4p+m, so om[p] maps to out rows 4p..4p+3 —
                    # a 12 KB/partition contiguous write.
                    out_eng.dma_start(out=dst, in_=om)

    nc.compile()
    return nc


def _prep_in_maps(hidden, word_ids):
    hidden = np.ascontiguousarray(np.asarray(hidden), dtype=np.float32).reshape(B, S, H)
    wid = np.ascontiguousarray(np.asarray(word_ids), dtype=np.int32).reshape(B, S)

    # Per-word piece counts -> 1/max(count,1).
    counts = np.zeros((B, T), np.int64)
    rows = np.repeat(np.arange(B), S)
    np.add.at(counts, (rows, wid.reshape(-1)), 1)
    recip = (1.0 / np.maximum(counts, 1)).astype(np.float32)  # [B, T]

    in_maps = []
    for i in range(N_CORES):
        sl = slice(i * B_LOC, (i + 1) * B_LOC)
        hs = np.ascontiguousarray(hidden[sl])
        ws = wid[sl]
        aux = np.ones((P, B_LOC, 2 * NM), np.float32)
        # aux[p, b, j] = wid[b, 128j+p]   (piece s = 128j + p)
        aux[:, :, :NM] = ws.reshape(B_LOC, J, P).transpose(2, 0, 1)
        # aux[p, b, 4+m] = recip[b, 4p+m]  (p < 100)
        aux[:MW, :, NM:] = recip[sl].reshape(B_LOC, MW, NM).transpose(1, 0, 2)
        in_maps.append({"hidden": hs, "aux_pb": np.ascontiguousarray(aux)})
    return in_maps


def run(hidden, word_ids, trace=False, **trace_kwargs):
    from concourse import bass_utils

    if "nc" not in _CACHED:
        _CACHED["nc"] = build_program()
    nc = _CACHED["nc"]
    in_maps = _prep_in_maps(hidden, word_ids)
    res = bass_utils.run_bass_kernel_spmd(
        nc, in_maps, core_ids=list(range(N_CORES)), trace=trace, **trace_kwargs
    )
    out = np.concatenate([res.results[i]["out"] for i in range(N_CORES)], axis=0)
    return out.astype(np.float32, copy=False), res


def kernel(hidden, word_ids, num_tokens=None, **_unused):
    out, _ = run(hidden, word_ids, trace=False)
    return out


# revision 18
# speedup vs baseline: 2.0717x; 1.1204x over previous
"""Segment mean-pool (BERT lattice embedding) Trainium2 Bass kernel.

Full-input contract: kernel(hidden[64,512,768] f32, word_ids[64,512] i32,
num_tokens=400) -> [64,400,768] f32.

Strategy: data-parallel over batch across 8 NeuronCores (8 samples each).
Per sample b the ragged segment mean  out[t] = mean_{s: wid[s]==t} hidden[s]
is computed as a matmul on the PE array:

    A_T[s, t] = (word_ids[b, s] == t)            one-hot, built on-device
    psum[t, :] = sum_j A_T[j-chunk].T @ hidden[b, j-chunk]
    out[t, h] = psum[t, h] * recip[b, t]         recip = 1/max(count,1)

All matmuls run in float32r (FP22-truncated fp32): full PE rate at even
N>=256, ~2e-4 relative error, and no dtype casts of the 100 MB activation
tensor.  (fp16/bf16 would halve the LDWEIGHTS time that paces the PE,
but the required f32->16-bit casts are ~28 us of ACT/DVE work that
starves the PSUM->SBUF->DMA drain those engines also carry — measured
net loss every time.  SWDGE can cast inside the DMA but its Q7
descriptor generator is ~7x too slow for this stream.)

The per-word piece counts (reciprocals) are derived on host from the
128 KB word_ids index tensor — index-side preprocessing, like the shard
layout transform; all heavy data stays on device.

Layouts are chosen for contiguous DMA descriptors and a cheap PE mix:
  - pieces:  partition p holds s = 128j+p -> 3 KB/partition descriptors
    (segment-sum is invariant to how s is split into K-chunks)
  - words:   M-chunks {128,128,128,16}: the 16-wide runt's LDWEIGHTS is
    ~2x cheaper, and the LAST output write per sample is tiny, so the
    drain tail is short.

DMA plan (kernel is HBM-bound: 12.6 MB in + 9.8 MB out per core at a
~415 GB/s practical per-core ceiling = ~54 us of unavoidable streaming):
  - one merged aux tensor (word ids + reciprocals, 256 B/partition) at
    the head of the sync ring — NOT two tiny-packet transfers;
  - all hidden prefetches on the sync HWDGE ring (sample 0 split per
    j-chunk so the first accumulation starts as soon as chunk 0 lands);
  - per-m-chunk output DMAs alternate between the scalar and sync HWDGE
    rings: the sync-ring ones queue behind the input stream (harmless —
    HBM is saturated by inputs then anyway) and the output-only drain
    phase at the end runs on BOTH descriptor generators instead of
    being serialized on one.
"""

import numpy as np

B, S, H, T = 64, 512, 768, 400
N_CORES = 8
B_LOC = B // N_CORES  # samples per core
P = 128
J = S // P  # contraction chunks per sample
N0 = 384  # h-chunk split: two equal psum banks, balances the scale engines
M_CHUNKS = [(0, 128), (128, 128), (256, 128), (384, T - 384)]  # (t0, mw)
NM = len(M_CHUNKS)

_CACHED = {}


def build_program():
    """Build + compile the single-core Bass program (same NEFF on all cores)."""
    import concourse.bass as bass  # noqa: F401
    import concourse.mybir as mybir
    import concourse.tile as tile
    from concourse import bacc

    nc = bacc.Bacc(
        "TRN2",
        target_bir_lowering=False,
        debug=False,
        enable_asserts=False,
        num_devices=N_CORES,
    )
    f32 = mybir.dt.float32
    f32r = mybir.dt.float32r

    hidden_t = nc.dram_tensor("hidden", [B_LOC, S, H], f32r, kind="ExternalInput").ap()
    # aux[p, b, 0:4] = word_ids[b, 128j+p] (fp32; values < 400 exact), the
    # per-partition scalar for piece-chunk j.  aux[p, b, 4:8] =
    # 1/max(count,1) for word t = 128m + p (t >= 400 padded with 1.0).
    aux_t = nc.dram_tensor("aux_pb", [P, B_LOC, 2 * NM], f32, kind="ExternalInput").ap()
    out_t = nc.dram_tensor("out", [B_LOC, T, H], f32, kind="ExternalOutput").ap()

    with tile.TileContext(nc) as tc:
        with tc.tile_pool(name="const", bufs=1) as const_pool, \
             tc.tile_pool(name="hidp", bufs=B_LOC) as hid_pool, \
             tc.tile_pool(name="aTp", bufs=3) as aT_pool, \
             tc.tile_pool(name="outp", bufs=8) as out_pool, \
             tc.tile_pool(name="psum", bufs=4, space="PSUM") as psum_pool:

            aux_sb = const_pool.tile([P, B_LOC, 2 * NM], f32, name="aux_sb")
            nc.sync.dma_start(out=aux_sb, in_=aux_t)

            iota_t = const_pool.tile([P, T], f32, name="iota_t")
            nc.gpsimd.iota(
                iota_t,
                pattern=[[1, T]],
                base=0,
                channel_multiplier=0,
                allow_small_or_imprecise_dtypes=True,
            )

            # Prefetch the whole input shard up front (fits in SBUF): the
            # input queue streams back-to-back from t=0 and compute is never
            # input-starved. One DMA per sample; 3 KB descriptors measured
            # faster end-to-end than 12 KB ones.
            hids = []
            for b in range(B_LOC):
                hid = hid_pool.tile([P, J, H], f32r, name=f"hid{b}", tag="hid")
                src = hidden_t[b].rearrange("(j p) h -> p j h", p=P)
                if b == 0:
                    # First sample split per j-chunk so the first accumulation
                    # can start as soon as chunk 0 lands.
                    for j in range(J):
                        nc.sync.dma_start(out=hid[:, j, :], in_=src[:, j, :])
                else:
                    nc.sync.dma_start(out=hid, in_=src)
                hids.append(hid)

            ci = 0  # global output-chunk counter for ring alternation
            for b in range(B_LOC):
                hid = hids[b]
                aT = aT_pool.tile([P, J, T], f32r, name="aT", tag="aT")
                for j in range(J):
                    nc.vector.tensor_scalar(
                        aT[:, j, :],
                        iota_t,
                        aux_sb[:, b, j : j + 1],
                        None,
                        op0=mybir.AluOpType.is_equal,
                    )
                for mi, (t0, mw) in enumerate(M_CHUNKS):
                    ps0 = psum_pool.tile([P, N0], f32, name="ps0", tag="ps0")
                    ps1 = psum_pool.tile([P, H - N0], f32, name="ps1", tag="ps1")
                    for j in range(J):
                        nc.tensor.matmul(
                            ps0[:mw],
                            aT[:, j, t0 : t0 + mw],
                            hid[:, j, 0:N0],
                            start=(j == 0),
                            stop=(j == J - 1),
                        )
                    for j in range(J):
                        nc.tensor.matmul(
                            ps1[:mw],
                            aT[:, j, t0 : t0 + mw],
                            hid[:, j, N0:H],
                            start=(j == 0),
                            stop=(j == J - 1),
                        )

                    rec = aux_sb[:, b, NM + mi : NM + mi + 1]
                    om = out_pool.tile([P, H], f32, name="om", tag="om")
                    # out = psum * (1/count): ACT and DVE each take one chunk,
                    # both read PSUM directly.
                    nc.scalar.mul(om[:mw, 0:N0], ps0[:mw], rec[:mw])
                    nc.vector.tensor_scalar_mul(om[:mw, N0:H], ps1[:mw], rec[:mw])
                    # Per-m-chunk output DMA right after its scale, rings
                    # alternating so the drain phase uses both HWDGE DGEs.
                    eng = nc.scalar if ci % 2 == 0 else nc.sync
                    ci += 1
                    eng.dma_start(out=out_t[b, t0 : t0 + mw], in_=om[:mw])

    nc.compile()
    return nc


def _prep_in_maps(hidden, word_ids):
    hidden = np.ascontiguousarray(np.asarray(hidden), dtype=np.float32).reshape(B, S, H)
    wid = np.ascontiguousarray(np.asarray(word_ids), dtype=np.int32).reshape(B, S)

    # Per-word piece counts -> 1/max(count,1), padded to 512 words.
    counts = np.zeros((B, P * NM), np.int64)
    rows = np.repeat(np.arange(B), S)
    np.add.at(counts, (rows, wid.reshape(-1)), 1)
    recip = (1.0 / np.maximum(counts, 1)).astype(np.float32)  # [B, 512]

    in_maps = []
    for i in range(N_CORES):
        sl = slice(i * B_LOC, (i + 1) * B_LOC)
        hs = np.ascontiguousarray(hidden[sl])
        ws = wid[sl]
        aux = np.ones((P, B_LOC, 2 * NM), np.float32)
        # aux[p, b, j] = wid[b, 128j+p]
        aux[:, :, :NM] = ws.reshape(B_LOC, J, P).transpose(2, 0, 1)
        # aux[p, b, 4+m] = recip[b, 128m+p]
        aux[:, :, NM:] = recip[sl].reshape(B_LOC, NM, P).transpose(2, 0, 1)
        in_maps.append({"hidden": hs, "aux_pb": np.ascontiguousarray(aux)})
    return in_maps


def run(hidden, word_ids, trace=False, **trace_kwargs):
    from concourse import bass_utils

    if "nc" not in _CACHED:
        _CACHED["nc"] = build_program()
    nc = _CACHED["nc"]
    in_maps = _prep_in_maps(hidden, word_ids)
    res = bass_utils.run_bass_kernel_spmd(
        nc, in_maps, core_ids=list(range(N_CORES)), trace=trace, **trace_kwargs
    )
    out = np.concatenate([res.results[i]["out"] for i in range(N_CORES)], axis=0)
    return out.astype(np.float32, copy=False), res


def kernel(hidden, word_ids, num_tokens=None, **_unused):
    out, _ = run(hidden, word_ids, trace=False)
    return out


# revision 22
# speedup vs baseline: 2.3348x; 1.1270x over previous
"""Segment mean-pool (BERT lattice embedding) Trainium2 Bass kernel.

Full-input contract: kernel(hidden[64,512,768] f32, word_ids[64,512] i32,
num_tokens=400) -> [64,400,768] f32.

Strategy: data-parallel over batch across 8 NeuronCores (8 samples each).
Per sample b the ragged segment mean  out[t] = mean_{s: wid[s]==t} hidden[s]
is computed as a matmul on the PE array:

    A_T[s, t] = (word_ids[b, s] == t)            one-hot, built on-device
    psum[t, :] = sum_j A_T[j-chunk].T @ hidden[b, j-chunk]
    out[t, h] = psum[t, h] * recip[b, t]         recip = 1/max(count,1)

All matmuls run in float32r (FP22-truncated fp32): full PE rate at even
N>=256, ~2e-4 relative error, and no dtype casts of the 100 MB activation
tensor.  (fp16/bf16 would halve the LDWEIGHTS time that paces the PE,
but the required f32->16-bit casts are ~28 us of ACT/DVE work that
starves the PSUM->SBUF->DMA drain those engines also carry — measured
net loss every time.  SWDGE can cast inside the DMA but its Q7
descriptor generator is ~7x too slow for this stream.)

The per-word piece counts (reciprocals) are derived on host from the
128 KB word_ids index tensor — index-side preprocessing, like the shard
layout transform; all heavy data stays on device.

Layouts are chosen for contiguous DMA descriptors and a cheap PE mix:
  - pieces:  partition p holds s = 128j+p -> 3 KB/partition descriptors
    (segment-sum is invariant to how s is split into K-chunks)
  - words:   M-chunks {128,128,128,16}: the 16-wide runt's LDWEIGHTS is
    ~2x cheaper, and the LAST output write per sample is tiny, so the
    drain tail is short.

DMA plan (kernel is HBM-bound: 12.6 MB in + 9.8 MB out per core at a
~415 GB/s practical per-core ceiling = ~54 us of unavoidable streaming):
  - one merged aux tensor (word ids + reciprocals, 256 B/partition) at
    the head of the sync ring — NOT two tiny-packet transfers;
  - all hidden prefetches on the sync HWDGE ring (sample 0 split per
    j-chunk so the first accumulation starts as soon as chunk 0 lands);
  - ALL output DMAs go on the sync ring, BEHIND the inputs: ring FIFO
    guarantees the input stream runs solo at ~410 GB/s (done by ~40 us,
    so the PE is never input-starved — outputs sharing HBM mid-phase
    measurably starves the PE for ~11 us around samples 4-5), while
    scaled chunks pile up in a deep om buffer (~20 x 3 KB/partition)
    and then drain at ring max.  Total = input-solo + output-drain
    lands within ~1 us of the HBM floor, which interleaving cannot
    beat anyway.
"""

import numpy as np

B, S, H, T = 64, 512, 768, 400
N_CORES = 8
B_LOC = B // N_CORES  # samples per core
P = 128
J = S // P  # contraction chunks per sample
N0 = 384  # h-chunk split: two equal psum banks, balances the scale engines
M_CHUNKS = [(0, 128), (128, 128), (256, 128), (384, T - 384)]  # (t0, mw)
NM = len(M_CHUNKS)

_CACHED = {}


def build_program():
    """Build + compile the single-core Bass program (same NEFF on all cores)."""
    import concourse.bass as bass  # noqa: F401
    import concourse.mybir as mybir
    import concourse.tile as tile
    from concourse import bacc

    nc = bacc.Bacc(
        "TRN2",
        target_bir_lowering=False,
        debug=False,
        enable_asserts=False,
        num_devices=N_CORES,
    )
    f32 = mybir.dt.float32
    f32r = mybir.dt.float32r

    hidden_t = nc.dram_tensor("hidden", [B_LOC, S, H], f32r, kind="ExternalInput").ap()
    # aux[p, b, 0:4] = word_ids[b, 128j+p] (fp32; values < 400 exact), the
    # per-partition scalar for piece-chunk j.  aux[p, b, 4:8] =
    # 1/max(count,1) for word t = 128m + p (t >= 400 padded with 1.0).
    aux_t = nc.dram_tensor("aux_pb", [P, B_LOC, 2 * NM], f32, kind="ExternalInput").ap()
    out_t = nc.dram_tensor("out", [B_LOC, T, H], f32, kind="ExternalOutput").ap()

    with tile.TileContext(nc) as tc:
        with tc.tile_pool(name="const", bufs=1) as const_pool, \
             tc.tile_pool(name="hidp", bufs=B_LOC) as hid_pool, \
             tc.tile_pool(name="aTp", bufs=3) as aT_pool, \
             tc.tile_pool(name="outp", bufs=20) as out_pool, \
             tc.tile_pool(name="psum", bufs=4, space="PSUM") as psum_pool:

            aux_sb = const_pool.tile([P, B_LOC, 2 * NM], f32, name="aux_sb")
            nc.sync.dma_start(out=aux_sb, in_=aux_t)

            iota_t = const_pool.tile([P, T], f32, name="iota_t")
            nc.gpsimd.iota(
                iota_t,
                pattern=[[1, T]],
                base=0,
                channel_multiplier=0,
                allow_small_or_imprecise_dtypes=True,
            )

            # Prefetch the whole input shard up front (fits in SBUF): the
            # input queue streams back-to-back from t=0 and compute is never
            # input-starved. One DMA per sample; 3 KB descriptors measured
            # faster end-to-end than 12 KB ones.
            hids = []
            for b in range(B_LOC):
                hid = hid_pool.tile([P, J, H], f32r, name=f"hid{b}", tag="hid")
                src = hidden_t[b].rearrange("(j p) h -> p j h", p=P)
                if b == 0:
                    # First sample split per j-chunk so the first accumulation
                    # can start as soon as chunk 0 lands.
                    for j in range(J):
                        nc.sync.dma_start(out=hid[:, j, :], in_=src[:, j, :])
                else:
                    nc.sync.dma_start(out=hid, in_=src)
                hids.append(hid)

            for b in range(B_LOC):
                hid = hids[b]
                aT = aT_pool.tile([P, J, T], f32r, name="aT", tag="aT")
                for j in range(J):
                    nc.vector.tensor_scalar(
                        aT[:, j, :],
                        iota_t,
                        aux_sb[:, b, j : j + 1],
                        None,
                        op0=mybir.AluOpType.is_equal,
                    )
                for mi, (t0, mw) in enumerate(M_CHUNKS):
                    ps0 = psum_pool.tile([P, N0], f32, name="ps0", tag="ps0")
                    ps1 = psum_pool.tile([P, H - N0], f32, name="ps1", tag="ps1")
                    for j in range(J):
                        nc.tensor.matmul(
                            ps0[:mw],
                            aT[:, j, t0 : t0 + mw],
                            hid[:, j, 0:N0],
                            start=(j == 0),
                            stop=(j == J - 1),
                        )
                    for j in range(J):
                        nc.tensor.matmul(
                            ps1[:mw],
                            aT[:, j, t0 : t0 + mw],
                            hid[:, j, N0:H],
                            start=(j == 0),
                            stop=(j == J - 1),
                        )

                    rec = aux_sb[:, b, NM + mi : NM + mi + 1]
                    om = out_pool.tile([P, H], f32, name="om", tag="om")
                    # out = psum * (1/count): ACT and DVE each take one chunk,
                    # both read PSUM directly.
                    nc.scalar.mul(om[:mw, 0:N0], ps0[:mw], rec[:mw])
                    nc.vector.tensor_scalar_mul(om[:mw, N0:H], ps1[:mw], rec[:mw])
                    # Per-m-chunk output DMA, issued as soon as its scale is
                    # done — data flows once the sync ring finishes the
                    # input prefetch.
                    nc.sync.dma_start(out=out_t[b, t0 : t0 + mw], in_=om[:mw])

    nc.compile()
    return nc


def _prep_in_maps(hidden, word_ids):
    hidden = np.ascontiguousarray(np.asarray(hidden), dtype=np.float32).reshape(B, S, H)
    wid = np.ascontiguousarray(np.asarray(word_ids), dtype=np.int32).reshape(B, S)

    # Per-word piece counts -> 1/max(count,1), padded to 512 words.
    counts = np.zeros((B, P * NM), np.int64)
    rows = np.repeat(np.arange(B), S)
    np.add.at(counts, (rows, wid.reshape(-1)), 1)
    recip = (1.0 / np.maximum(counts, 1)).astype(np.float32)  # [B, 512]

    in_maps = []
    for i in range(N_CORES):
        sl = slice(i * B_LOC, (i + 1) * B_LOC)
        hs = np.ascontiguousarray(hidden[sl])
        ws = wid[sl]
        aux = np.ones((P, B_LOC, 2 * NM), np.float32)
        # aux[p, b, j] = wid[b, 128j+p]
        aux[:, :, :NM] = ws.reshape(B_LOC, J, P).transpose(2, 0, 1)
        # aux[p, b, 4+m] = recip[b, 128m+p]
        aux[:, :, NM:] = recip[sl].reshape(B_LOC, NM, P).transpose(2, 0, 1)
        in_maps.append({"hidden": hs, "aux_pb": np.ascontiguousarray(aux)})
    return in_maps


def run(hidden, word_ids, trace=False, **trace_kwargs):
    from concourse import bass_utils

    if "nc" not in _CACHED:
        _CACHED["nc"] = build_program()
    nc = _CACHED["nc"]
    in_maps = _prep_in_maps(hidden, word_ids)
    res = bass_utils.run_bass_kernel_spmd(
        nc, in_maps, core_ids=list(range(N_CORES)), trace=trace, **trace_kwargs
    )
    out = np.concatenate([res.results[i]["out"] for i in range(N_CORES)], axis=0)
    return out.astype(np.float32, copy=False), res


def kernel(hidden, word_ids, num_tokens=None, **_unused):
    out, _ = run(hidden, word_ids, trace=False)
    return out
